# revision 10
# baseline (speedup 1.0000x reference)
"""Multi-head attention (B=4, T=2048, D=1024, H=16) on 8 TRN2 NeuronCores.

Sharding: core c handles batch b = c//2 and head-half hh = c%2 (8 heads,
512 of the 1024 channel dims). Each core computes its half of the head
outputs and a row-sharded output projection, producing a partial
[T, D] output. Host unshard: out[b] = partial[2b] + partial[2b+1]
+ b_o + b_v @ w_o.T (the value-bias contribution commutes through
attention because softmax rows sum to 1).

All matmul operands are bf16 (hosts converts inputs); PSUM accumulates
fp32. Score PSUM is double-buffered ([128, 2x512] tiles) so the PE
computes the next group's scores while ACT runs exp on the previous
one, keeping the PE dense enough to hold its max p-state.
"""

from contextlib import ExitStack

import ml_dtypes
import numpy as np

import concourse.bass as bass
import concourse.mybir as mybir
import concourse.tile as tile
from concourse import bacc
from concourse.bass_utils import run_bass_kernel_spmd

B, T, D = 4, 2048, 1024
H = 16
DH = 64  # head dim
HALF = 512  # channels per core (8 heads)
N_CORES = 8

F32 = mybir.dt.float32
BF16 = mybir.dt.bfloat16
NPBF16 = ml_dtypes.bfloat16

TB = 512  # t-block for moving operands
NTB = T // TB  # 4
KB = 128  # contraction block
NKB = D // KB  # 8
NJB = HALF // KB  # 4 j-blocks of the half
NTK = T // KB  # 16 tk blocks


def build_kernel():
    nc = bacc.Bacc(
        "TRN2", target_bir_lowering=False, debug=False, num_devices=N_CORES
    )
    xqT = nc.dram_tensor("xqT", [D, T], BF16, kind="ExternalInput").ap()
    xkT = nc.dram_tensor("xkT", [D, T], BF16, kind="ExternalInput").ap()
    xvT = nc.dram_tensor("xvT", [D, T], BF16, kind="ExternalInput").ap()
    wqT = nc.dram_tensor("wqT", [D, HALF], BF16, kind="ExternalInput").ap()
    wkT = nc.dram_tensor("wkT", [D, HALF], BF16, kind="ExternalInput").ap()
    wvT = nc.dram_tensor("wvT", [D, HALF], BF16, kind="ExternalInput").ap()
    woT = nc.dram_tensor("woT", [HALF, D], BF16, kind="ExternalInput").ap()
    bq = nc.dram_tensor("bq", [HALF, 1], F32, kind="ExternalInput").ap()
    bk = nc.dram_tensor("bk", [HALF, 1], F32, kind="ExternalInput").ap()
    partial = nc.dram_tensor("partial", [T, D], F32, kind="ExternalOutput").ap()

    with tile.TileContext(nc) as tc, ExitStack() as ctx:
        p_const = ctx.enter_context(tc.tile_pool(name="const", bufs=1))
        p_kt = ctx.enter_context(tc.tile_pool(name="kt", bufs=NJB))
        p_qt = ctx.enter_context(tc.tile_pool(name="qt", bufs=NJB))
        p_v = ctx.enter_context(tc.tile_pool(name="v", bufs=NTK))
        p_xs = ctx.enter_context(tc.tile_pool(name="xs", bufs=8))
        p_ex = ctx.enter_context(tc.tile_pool(name="ex", bufs=6))
        p_ot = ctx.enter_context(tc.tile_pool(name="ot", bufs=8))
        p_rc = ctx.enter_context(tc.tile_pool(name="rc", bufs=6))
        p_st = ctx.enter_context(tc.tile_pool(name="st", bufs=2))
        # PSUM: scores 2x2 banks + av 2x1 + proj 2x1 = 8 banks
        p_sc = ctx.enter_context(tc.tile_pool(name="sc", bufs=2, space="PSUM"))
        p_av = ctx.enter_context(tc.tile_pool(name="av", bufs=2, space="PSUM"))
        p_pj = ctx.enter_context(tc.tile_pool(name="pj", bufs=2, space="PSUM"))

        # ---- constants ----
        w_q = p_const.tile([KB, NKB, HALF], BF16, tag="wq")
        nc.sync.dma_start(w_q[:], wqT.rearrange("(kb p) j -> p kb j", p=KB))
        w_k = p_const.tile([KB, NKB, HALF], BF16, tag="wk")
        nc.sync.dma_start(w_k[:], wkT.rearrange("(kb p) j -> p kb j", p=KB))
        w_v = p_const.tile([KB, NKB, HALF], BF16, tag="wv")
        nc.sync.dma_start(w_v[:], wvT.rearrange("(kb p) j -> p kb j", p=KB))
        w_o = p_const.tile([KB, NJB, D], BF16, tag="wo")
        nc.sync.dma_start(w_o[:], woT.rearrange("(jb p) n -> p jb n", p=KB))
        b_q = p_const.tile([KB, NJB], F32, tag="bq")
        nc.sync.dma_start(b_q[:], bq.rearrange("(jb p) one -> p (jb one)", p=KB))
        b_k = p_const.tile([KB, NJB], F32, tag="bk")
        nc.sync.dma_start(b_k[:], bk.rearrange("(jb p) one -> p (jb one)", p=KB))

        # ---- K^T / Q^T projections: {kt,qt}[jb] is [128 (j), T] bf16 ----
        kt_tiles = [p_kt.tile([KB, T], BF16, tag="kt", name=f"kt{j}") for j in range(NJB)]
        qt_tiles = [p_qt.tile([KB, T], BF16, tag="qt", name=f"qt{j}") for j in range(NJB)]
        for x_in, w_in, b_in, dst in (
            (xkT, w_k, b_k, kt_tiles),
            (xqT, w_q, b_q, qt_tiles),
        ):
            for tb in range(NTB):
                # 4 accumulators per tb: one 2-slot sc tile + two 1-bank pj
                # tiles; rings leave a full tb between reuse so the bias-add
                # drains overlap the next tb's matmuls.
                ps = p_sc.tile([KB, 2, TB], F32, tag="sc", name=f"psp{tb}")
                pos = [
                    p_pj.tile([KB, TB], F32, tag="po", name=f"pop{tb}_{u}")
                    for u in range(2)
                ]
                targets = [ps[:, 0, :], ps[:, 1, :], pos[0][:], pos[1][:]]
                for kb in range(NKB):
                    xt = p_xs.tile([KB, TB], BF16, tag="xs")
                    nc.sync.dma_start(
                        xt[:],
                        x_in[kb * KB : (kb + 1) * KB, tb * TB : (tb + 1) * TB],
                    )
                    for jb in range(NJB):
                        nc.tensor.matmul(
                            targets[jb],
                            w_in[:, kb, jb * KB : (jb + 1) * KB],
                            xt[:],
                            start=(kb == 0),
                            stop=(kb == NKB - 1),
                        )
                for jb in range(NJB):
                    nc.vector.tensor_scalar_add(
                        dst[jb][:, tb * TB : (tb + 1) * TB],
                        targets[jb],
                        b_in[:, jb : jb + 1],
                    )

        # ---- V projection (natural layout): V[tk] is [128 (t), 8 (h), 65] ----
        # column 64 of each head is 1.0: the AV matmul then accumulates the
        # softmax denominator in psum row 64 for free.
        v_tiles = [
            p_v.tile([KB, H // 2, DH + 1], BF16, tag="v", name=f"v{j}")
            for j in range(NTK)
        ]
        for t in range(NTK):
            nc.vector.memset(v_tiles[t][:, :, DH : DH + 1], 1.0)
        for tb in range(NTB):
            ps = p_sc.tile([KB, 2, TB], F32, tag="sc", name=f"psv{tb}")
            pos = [
                p_pj.tile([KB, TB], F32, tag="po", name=f"pov{tb}_{u}")
                for u in range(2)
            ]
            targets = [ps[:, 0, :], ps[:, 1, :], pos[0][:], pos[1][:]]
            for kb in range(NKB):
                xt = p_xs.tile([KB, TB], BF16, tag="xs")
                nc.sync.dma_start(
                    xt[:], xvT[kb * KB : (kb + 1) * KB, tb * TB : (tb + 1) * TB]
                )
                for ts in range(4):
                    nc.tensor.matmul(
                        targets[ts],
                        xt[:, ts * KB : (ts + 1) * KB],
                        w_v[:, kb, :],
                        start=(kb == 0),
                        stop=(kb == NKB - 1),
                    )
            for ts in range(4):
                nc.vector.tensor_copy(
                    v_tiles[tb * 4 + ts][:, :, 0:DH],
                    targets[ts].rearrange("p (h d) -> p h d", d=DH),
                )

        # ---- per t-block: attention + out-projection ----
        # The out-projection for t-block tq is emitted interleaved into the
        # score groups of t-block tq+1 so the PE never stalls waiting for the
        # last head-pair's softmax normalization.
        def emit_po_chain(ot_tiles, tq, nb, ts):
            po = p_pj.tile([KB, TB], F32, tag="po", name=f"po{tq}_{nb}_{ts}")
            for jp in range(NJB):
                nc.tensor.matmul(
                    po[:],
                    ot_tiles[jp][:, ts * KB : (ts + 1) * KB],
                    w_o[:, jp, nb * TB : (nb + 1) * TB],
                    start=(jp == 0),
                    stop=(jp == NJB - 1),
                )
            st = p_st.tile([KB, TB], F32, tag="st", name=f"st{tq}_{nb}_{ts}")
            nc.vector.tensor_copy(st[:], po[:])
            nc.sync.dma_start(
                partial[
                    tq * TB + ts * KB : tq * TB + (ts + 1) * KB,
                    nb * TB : (nb + 1) * TB,
                ],
                st[:],
            )

        pending = []  # deferred out-proj chains from the previous t-block
        for tq in range(NTB):
            ot_tiles = [
                p_ot.tile([KB, TB], BF16, tag="ot", name=f"ot{tq}_{j}")
                for j in range(NJB)
            ]
            gctr = 0
            for jp in range(NJB):  # head pair (2*jp, 2*jp+1)
                avs = [
                    p_av.tile([DH + 1, TB], F32, tag="av", name=f"av{i}")
                    for i in range(2)
                ]
                for g in range(NTK):
                    sc = p_sc.tile([KB, 2, TB], F32, tag="sc")
                    for i in range(2):
                        nc.tensor.matmul(
                            sc[:, i, :],
                            kt_tiles[jp][i * DH : (i + 1) * DH, g * KB : (g + 1) * KB],
                            qt_tiles[jp][i * DH : (i + 1) * DH, tq * TB : (tq + 1) * TB],
                            start=True,
                            stop=True,
                        )
                    ex = p_ex.tile([KB, 2, TB], BF16, tag="ex")
                    nc.scalar.activation(
                        ex[:], sc[:], mybir.ActivationFunctionType.Exp, scale=0.125
                    )
                    for i in range(2):
                        nc.tensor.matmul(
                            avs[i][:],
                            v_tiles[g][:, 2 * jp + i, :],
                            ex[:, i, :],
                            start=(g == 0),
                            stop=(g == NTK - 1),
                        )
                    gctr += 1
                    if pending and gctr >= 6 and gctr % 2 == 0:
                        emit_po_chain(*pending.pop(0))
                for i in range(2):
                    # copy the whole AV psum (including the denominator row 64)
                    # to SBUF immediately so the psum bank frees for the next
                    # head pair; normalize from the SBUF copy.
                    asb = p_rc.tile([DH + 1, TB], F32, tag="asb")
                    nc.vector.tensor_copy(asb[:], avs[i][:])
                    bc = p_rc.tile([DH, TB], F32, tag="bc")
                    nc.gpsimd.dma_start(
                        bc[:],
                        asb[DH : DH + 1, None, :].broadcast_to([1, DH, TB]),
                    )
                    rc2 = p_rc.tile([DH, TB], F32, tag="rc2")
                    nc.vector.reciprocal_approx_fast(rc2[:], bc[:])
                    if i == 0:
                        nc.vector.tensor_mul(ot_tiles[jp][0:DH, :], asb[0:DH, :], rc2[:])
                    else:
                        # DVE can't shift partitions; stage then DMA into rows
                        # 64:128 (DMAs deferred to tq end, off the jp critical
                        # path of the shared DMA FIFO ring)
                        stg = p_rc.tile([DH, TB], BF16, tag="stg")
                        nc.vector.tensor_mul(stg[:], asb[0:DH, :], rc2[:])
                        nc.gpsimd.dma_start(ot_tiles[jp][DH : 2 * DH, :], stg[:])

            pending = [(ot_tiles, tq, nb, ts) for nb in range(2) for ts in range(4)]

        for args in pending:
            emit_po_chain(*args)

    nc.compile()
    return nc


def kernel(**inputs: np.ndarray) -> np.ndarray:
    query = np.asarray(inputs["query"], dtype=np.float32)
    key = np.asarray(inputs["key"], dtype=np.float32)
    value = np.asarray(inputs["value"], dtype=np.float32)
    w_q = np.asarray(inputs["w_q"], dtype=np.float32)
    b_q = np.asarray(inputs["b_q"], dtype=np.float32)
    w_k = np.asarray(inputs["w_k"], dtype=np.float32)
    b_k = np.asarray(inputs["b_k"], dtype=np.float32)
    w_v = np.asarray(inputs["w_v"], dtype=np.float32)
    b_v = np.asarray(inputs["b_v"], dtype=np.float32)
    w_o = np.asarray(inputs["w_o"], dtype=np.float32)
    b_o = np.asarray(inputs["b_o"], dtype=np.float32)

    nc = build_kernel()

    in_maps = []
    for c in range(N_CORES):
        b = c // 2
        hh = c % 2
        sl = slice(hh * HALF, (hh + 1) * HALF)
        in_maps.append(
            {
                "xqT": np.ascontiguousarray(query[b].T).astype(NPBF16),
                "xkT": np.ascontiguousarray(key[b].T).astype(NPBF16),
                "xvT": np.ascontiguousarray(value[b].T).astype(NPBF16),
                "wqT": np.ascontiguousarray(w_q[sl, :].T).astype(NPBF16),
                "wkT": np.ascontiguousarray(w_k[sl, :].T).astype(NPBF16),
                "wvT": np.ascontiguousarray(w_v[sl, :].T).astype(NPBF16),
                "woT": np.ascontiguousarray(w_o[:, sl].T).astype(NPBF16),
                "bq": np.ascontiguousarray(b_q[sl].reshape(HALF, 1)),
                "bk": np.ascontiguousarray(b_k[sl].reshape(HALF, 1)),
            }
        )

    res = run_bass_kernel_spmd(nc, in_maps, core_ids=list(range(N_CORES)))

    const_row = (b_v[None, :] @ w_o.T + b_o[None, :]).astype(np.float32)
    out = np.empty((B, T, D), dtype=np.float32)
    for b in range(B):
        out[b] = res.results[2 * b]["partial"] + res.results[2 * b + 1]["partial"]
        out[b] += const_row
    return out


# revision 11
# speedup vs baseline: 1.0566x; 1.0566x over previous
"""Multi-head attention (B=4, T=2048, D=1024, H=16) on 8 TRN2 NeuronCores.

Sharding: core c handles batch b = c//2 and head-half hh = c%2 (8 heads,
512 of the 1024 channel dims). Each core computes its half of the head
outputs and a row-sharded output projection, producing a partial
[T, D] output. Host unshard: out[b] = partial[2b] + partial[2b+1]
+ b_o + b_v @ w_o.T (the value-bias contribution commutes through
attention because softmax rows sum to 1).

All matmul operands are bf16 (hosts converts inputs); PSUM accumulates
fp32. Score PSUM is double-buffered ([128, 2x512] tiles) so the PE
computes the next group's scores while ACT runs exp on the previous
one, keeping the PE dense enough to hold its max p-state.
"""

from contextlib import ExitStack

import ml_dtypes
import numpy as np

import concourse.bass as bass
import concourse.mybir as mybir
import concourse.tile as tile
from concourse import bacc
from concourse.bass_utils import run_bass_kernel_spmd

B, T, D = 4, 2048, 1024
H = 16
DH = 64  # head dim
HALF = 512  # channels per core (8 heads)
N_CORES = 8

F32 = mybir.dt.float32
BF16 = mybir.dt.bfloat16
NPBF16 = ml_dtypes.bfloat16

TB = 512  # t-block for moving operands
NTB = T // TB  # 4
KB = 128  # contraction block
NKB = D // KB  # 8
NJB = HALF // KB  # 4 j-blocks of the half
NTK = T // KB  # 16 tk blocks


def build_kernel():
    nc = bacc.Bacc(
        "TRN2", target_bir_lowering=False, debug=False, num_devices=N_CORES
    )
    xqT = nc.dram_tensor("xqT", [D, T], BF16, kind="ExternalInput").ap()
    xkT = nc.dram_tensor("xkT", [D, T], BF16, kind="ExternalInput").ap()
    xvT = nc.dram_tensor("xvT", [D, T], BF16, kind="ExternalInput").ap()
    wqT = nc.dram_tensor("wqT", [D, HALF], BF16, kind="ExternalInput").ap()
    wkT = nc.dram_tensor("wkT", [D, HALF], BF16, kind="ExternalInput").ap()
    wvT = nc.dram_tensor("wvT", [D, HALF], BF16, kind="ExternalInput").ap()
    woT = nc.dram_tensor("woT", [HALF, D], BF16, kind="ExternalInput").ap()
    bq = nc.dram_tensor("bq", [HALF, 1], F32, kind="ExternalInput").ap()
    bk = nc.dram_tensor("bk", [HALF, 1], F32, kind="ExternalInput").ap()
    partial = nc.dram_tensor("partial", [T, D], F32, kind="ExternalOutput").ap()

    with tile.TileContext(nc) as tc, ExitStack() as ctx:
        p_const = ctx.enter_context(tc.tile_pool(name="const", bufs=1))
        p_kt = ctx.enter_context(tc.tile_pool(name="kt", bufs=NJB))
        p_qt = ctx.enter_context(tc.tile_pool(name="qt", bufs=NJB))
        p_v = ctx.enter_context(tc.tile_pool(name="v", bufs=NTK))
        p_xs = ctx.enter_context(tc.tile_pool(name="xs", bufs=8))
        p_ex = ctx.enter_context(tc.tile_pool(name="ex", bufs=6))
        p_ot = ctx.enter_context(tc.tile_pool(name="ot", bufs=8))
        p_rc = ctx.enter_context(tc.tile_pool(name="rc", bufs=6))
        p_st = ctx.enter_context(tc.tile_pool(name="st", bufs=2))
        # PSUM: scores 2x2 banks + av 2x1 + proj 2x1 = 8 banks
        p_sc = ctx.enter_context(tc.tile_pool(name="sc", bufs=2, space="PSUM"))
        p_av = ctx.enter_context(tc.tile_pool(name="av", bufs=2, space="PSUM"))
        p_pj = ctx.enter_context(tc.tile_pool(name="pj", bufs=2, space="PSUM"))

        # ---- constants ----
        w_q = p_const.tile([KB, NKB, HALF], BF16, tag="wq")
        nc.sync.dma_start(w_q[:], wqT.rearrange("(kb p) j -> p kb j", p=KB))
        w_k = p_const.tile([KB, NKB, HALF], BF16, tag="wk")
        nc.sync.dma_start(w_k[:], wkT.rearrange("(kb p) j -> p kb j", p=KB))
        w_v = p_const.tile([KB, NKB, HALF], BF16, tag="wv")
        nc.sync.dma_start(w_v[:], wvT.rearrange("(kb p) j -> p kb j", p=KB))
        w_o = p_const.tile([KB, NJB, D], BF16, tag="wo")
        nc.sync.dma_start(w_o[:], woT.rearrange("(jb p) n -> p jb n", p=KB))
        b_q = p_const.tile([KB, NJB], F32, tag="bq")
        nc.sync.dma_start(b_q[:], bq.rearrange("(jb p) one -> p (jb one)", p=KB))
        b_k = p_const.tile([KB, NJB], F32, tag="bk")
        nc.sync.dma_start(b_k[:], bk.rearrange("(jb p) one -> p (jb one)", p=KB))

        # ---- K^T / Q^T projections: {kt,qt}[jb] is [128 (j), T] bf16 ----
        kt_tiles = [p_kt.tile([KB, T], BF16, tag="kt", name=f"kt{j}") for j in range(NJB)]
        qt_tiles = [p_qt.tile([KB, T], BF16, tag="qt", name=f"qt{j}") for j in range(NJB)]
        for x_in, w_in, b_in, dst in (
            (xkT, w_k, b_k, kt_tiles),
            (xqT, w_q, b_q, qt_tiles),
        ):
            for tb in range(NTB):
                # 4 accumulators per tb: one 2-slot sc tile + two 1-bank pj
                # tiles; rings leave a full tb between reuse so the bias-add
                # drains overlap the next tb's matmuls.
                ps = p_sc.tile([KB, 2, TB], F32, tag="sc", name=f"psp{tb}")
                pos = [
                    p_pj.tile([KB, TB], F32, tag="po", name=f"pop{tb}_{u}")
                    for u in range(2)
                ]
                targets = [ps[:, 0, :], ps[:, 1, :], pos[0][:], pos[1][:]]
                for kb in range(NKB):
                    xt = p_xs.tile([KB, TB], BF16, tag="xs")
                    nc.sync.dma_start(
                        xt[:],
                        x_in[kb * KB : (kb + 1) * KB, tb * TB : (tb + 1) * TB],
                    )
                    for jb in range(NJB):
                        nc.tensor.matmul(
                            targets[jb],
                            w_in[:, kb, jb * KB : (jb + 1) * KB],
                            xt[:],
                            start=(kb == 0),
                            stop=(kb == NKB - 1),
                        )
                for jb in range(NJB):
                    nc.vector.tensor_scalar_add(
                        dst[jb][:, tb * TB : (tb + 1) * TB],
                        targets[jb],
                        b_in[:, jb : jb + 1],
                    )

        # ---- V projection (natural layout): V[tk] is [128 (t), 8 (h), 65] ----
        # column 64 of each head is 1.0: the AV matmul then accumulates the
        # softmax denominator in psum row 64 for free.
        v_tiles = [
            p_v.tile([KB, H // 2, DH + 1], BF16, tag="v", name=f"v{j}")
            for j in range(NTK)
        ]
        for t in range(NTK):
            nc.vector.memset(v_tiles[t][:, :, DH : DH + 1], 1.0)
        for tb in range(NTB):
            ps = p_sc.tile([KB, 2, TB], F32, tag="sc", name=f"psv{tb}")
            pos = [
                p_pj.tile([KB, TB], F32, tag="po", name=f"pov{tb}_{u}")
                for u in range(2)
            ]
            targets = [ps[:, 0, :], ps[:, 1, :], pos[0][:], pos[1][:]]
            for kb in range(NKB):
                xt = p_xs.tile([KB, TB], BF16, tag="xs")
                nc.sync.dma_start(
                    xt[:], xvT[kb * KB : (kb + 1) * KB, tb * TB : (tb + 1) * TB]
                )
                for ts in range(4):
                    nc.tensor.matmul(
                        targets[ts],
                        xt[:, ts * KB : (ts + 1) * KB],
                        w_v[:, kb, :],
                        start=(kb == 0),
                        stop=(kb == NKB - 1),
                    )
            for ts in range(4):
                nc.vector.tensor_copy(
                    v_tiles[tb * 4 + ts][:, :, 0:DH],
                    targets[ts].rearrange("p (h d) -> p h d", d=DH),
                )

        # ---- per t-block: attention + out-projection ----
        # The out-projection for t-block tq is emitted interleaved into the
        # score groups of t-block tq+1 so the PE never stalls waiting for the
        # last head-pair's softmax normalization.
        def make_po_ops(ot_tiles, tq):
            """One callable per out-proj matmul (4 per chain, 8 chains); the
            last op of each chain adds the psum->sbuf copy and output DMA.
            Popped one per score group so the PE cost (~0.4us) interleaves
            with the ACT-bound exp cadence instead of arriving as 1.7us
            monoliths."""
            ops = []
            for nb in range(2):
                for ts in range(4):
                    state = {}

                    def mk(jp, nb=nb, ts=ts, state=state):
                        def f():
                            if jp == 0:
                                state["po"] = p_pj.tile(
                                    [KB, TB], F32, tag="po", name=f"po{tq}_{nb}_{ts}"
                                )
                            nc.tensor.matmul(
                                state["po"][:],
                                ot_tiles[jp][:, ts * KB : (ts + 1) * KB],
                                w_o[:, jp, nb * TB : (nb + 1) * TB],
                                start=(jp == 0),
                                stop=(jp == NJB - 1),
                            )
                            if jp == NJB - 1:
                                st = p_st.tile(
                                    [KB, TB], F32, tag="st", name=f"st{tq}_{nb}_{ts}"
                                )
                                nc.vector.tensor_copy(st[:], state["po"][:])
                                nc.sync.dma_start(
                                    partial[
                                        tq * TB + ts * KB : tq * TB + (ts + 1) * KB,
                                        nb * TB : (nb + 1) * TB,
                                    ],
                                    st[:],
                                )

                        return f

                    for jp in range(NJB):
                        ops.append(mk(jp))
            return ops

        pending = []  # deferred out-proj ops from the previous t-block
        for tq in range(NTB):
            ot_tiles = [
                p_ot.tile([KB, TB], BF16, tag="ot", name=f"ot{tq}_{j}")
                for j in range(NJB)
            ]
            gctr = 0
            for jp in range(NJB):  # head pair (2*jp, 2*jp+1)
                avs = [
                    p_av.tile([DH + 1, TB], F32, tag="av", name=f"av{i}")
                    for i in range(2)
                ]
                for g in range(NTK):
                    sc = p_sc.tile([KB, 2, TB], F32, tag="sc")
                    for i in range(2):
                        nc.tensor.matmul(
                            sc[:, i, :],
                            kt_tiles[jp][i * DH : (i + 1) * DH, g * KB : (g + 1) * KB],
                            qt_tiles[jp][i * DH : (i + 1) * DH, tq * TB : (tq + 1) * TB],
                            start=True,
                            stop=True,
                        )
                    ex = p_ex.tile([KB, 2, TB], BF16, tag="ex")
                    nc.scalar.activation(
                        ex[:], sc[:], mybir.ActivationFunctionType.Exp, scale=0.125
                    )
                    for i in range(2):
                        nc.tensor.matmul(
                            avs[i][:],
                            v_tiles[g][:, 2 * jp + i, :],
                            ex[:, i, :],
                            start=(g == 0),
                            stop=(g == NTK - 1),
                        )
                    gctr += 1
                    if pending and gctr % 2 == 0:
                        pending.pop(0)()
                for i in range(2):
                    # copy the whole AV psum (including the denominator row 64)
                    # to SBUF immediately so the psum bank frees for the next
                    # head pair; normalize from the SBUF copy.
                    asb = p_rc.tile([DH + 1, TB], F32, tag="asb")
                    nc.vector.tensor_copy(asb[:], avs[i][:])
                    bc = p_rc.tile([DH, TB], F32, tag="bc")
                    nc.gpsimd.dma_start(
                        bc[:],
                        asb[DH : DH + 1, None, :].broadcast_to([1, DH, TB]),
                    )
                    rc2 = p_rc.tile([DH, TB], F32, tag="rc2")
                    nc.vector.reciprocal_approx_fast(rc2[:], bc[:])
                    if i == 0:
                        nc.vector.tensor_mul(ot_tiles[jp][0:DH, :], asb[0:DH, :], rc2[:])
                    else:
                        # DVE can't shift partitions; stage then DMA into rows
                        # 64:128 (DMAs deferred to tq end, off the jp critical
                        # path of the shared DMA FIFO ring)
                        stg = p_rc.tile([DH, TB], BF16, tag="stg")
                        nc.vector.tensor_mul(stg[:], asb[0:DH, :], rc2[:])
                        nc.gpsimd.dma_start(ot_tiles[jp][DH : 2 * DH, :], stg[:])

            pending = make_po_ops(ot_tiles, tq)

        for op in pending:
            op()

    nc.compile()
    return nc


def kernel(**inputs: np.ndarray) -> np.ndarray:
    query = np.asarray(inputs["query"], dtype=np.float32)
    key = np.asarray(inputs["key"], dtype=np.float32)
    value = np.asarray(inputs["value"], dtype=np.float32)
    w_q = np.asarray(inputs["w_q"], dtype=np.float32)
    b_q = np.asarray(inputs["b_q"], dtype=np.float32)
    w_k = np.asarray(inputs["w_k"], dtype=np.float32)
    b_k = np.asarray(inputs["b_k"], dtype=np.float32)
    w_v = np.asarray(inputs["w_v"], dtype=np.float32)
    b_v = np.asarray(inputs["b_v"], dtype=np.float32)
    w_o = np.asarray(inputs["w_o"], dtype=np.float32)
    b_o = np.asarray(inputs["b_o"], dtype=np.float32)

    nc = build_kernel()

    in_maps = []
    for c in range(N_CORES):
        b = c // 2
        hh = c % 2
        sl = slice(hh * HALF, (hh + 1) * HALF)
        in_maps.append(
            {
                "xqT": np.ascontiguousarray(query[b].T).astype(NPBF16),
                "xkT": np.ascontiguousarray(key[b].T).astype(NPBF16),
                "xvT": np.ascontiguousarray(value[b].T).astype(NPBF16),
                "wqT": np.ascontiguousarray(w_q[sl, :].T).astype(NPBF16),
                "wkT": np.ascontiguousarray(w_k[sl, :].T).astype(NPBF16),
                "wvT": np.ascontiguousarray(w_v[sl, :].T).astype(NPBF16),
                "woT": np.ascontiguousarray(w_o[:, sl].T).astype(NPBF16),
                "bq": np.ascontiguousarray(b_q[sl].reshape(HALF, 1)),
                "bk": np.ascontiguousarray(b_k[sl].reshape(HALF, 1)),
            }
        )

    res = run_bass_kernel_spmd(nc, in_maps, core_ids=list(range(N_CORES)))

    const_row = (b_v[None, :] @ w_o.T + b_o[None, :]).astype(np.float32)
    out = np.empty((B, T, D), dtype=np.float32)
    for b in range(B):
        out[b] = res.results[2 * b]["partial"] + res.results[2 * b + 1]["partial"]
        out[b] += const_row
    return out


# revision 21
# speedup vs baseline: 1.3097x; 1.2395x over previous
"""Multi-head attention (B=4, T=2048, D=1024, H=16) on 8 TRN2 NeuronCores.

Sharding: core c handles batch b = c//2 and head-half hh = c%2 (8 heads,
512 of the 1024 channel dims). Each core computes its half of the head
outputs and a row-sharded output projection, producing a partial
[T, D] output. Host unshard: out[b] = partial[2b] + partial[2b+1]
+ b_o + b_v @ w_o.T (the value-bias contribution commutes through
attention because softmax rows sum to 1).

All matmul operands are bf16 (hosts converts inputs); PSUM accumulates
fp32. Score PSUM is double-buffered ([128, 2x512] tiles) so the PE
computes the next group's scores while ACT runs exp on the previous
one, keeping the PE dense enough to hold its max p-state.
"""

from contextlib import ExitStack

import ml_dtypes
import numpy as np

import concourse.bass as bass
import concourse.mybir as mybir
import concourse.tile as tile
from concourse import bacc
from concourse.bass_utils import run_bass_kernel_spmd

B, T, D = 4, 2048, 1024
H = 16
DH = 64  # head dim
HALF = 512  # channels per core (8 heads)
N_CORES = 8

F32 = mybir.dt.float32
F32R = mybir.dt.float32r
BF16 = mybir.dt.bfloat16
NPBF16 = ml_dtypes.bfloat16

TB = 512  # t-block for moving operands
NTB = T // TB  # 4
KB = 128  # contraction block
NKB = D // KB  # 8
NJB = HALF // KB  # 4 j-blocks of the half
NTK = T // KB  # 16 tk blocks


def build_kernel():
    nc = bacc.Bacc(
        "TRN2", target_bir_lowering=False, debug=False, num_devices=N_CORES
    )
    xqT = nc.dram_tensor("xqT", [D, T], BF16, kind="ExternalInput").ap()
    xkT = nc.dram_tensor("xkT", [D, T], BF16, kind="ExternalInput").ap()
    xvT = nc.dram_tensor("xvT", [D, T], BF16, kind="ExternalInput").ap()
    wqT = nc.dram_tensor("wqT", [D, HALF], BF16, kind="ExternalInput").ap()
    wkT = nc.dram_tensor("wkT", [D, HALF], BF16, kind="ExternalInput").ap()
    wvT = nc.dram_tensor("wvT", [D, HALF], BF16, kind="ExternalInput").ap()
    woT = nc.dram_tensor("woT", [HALF, D], BF16, kind="ExternalInput").ap()
    bq = nc.dram_tensor("bq", [HALF, 1], F32, kind="ExternalInput").ap()
    bk = nc.dram_tensor("bk", [HALF, 1], F32, kind="ExternalInput").ap()
    partial = nc.dram_tensor("partial", [T, D], F32, kind="ExternalOutput").ap()

    with tile.TileContext(nc) as tc, ExitStack() as ctx:
        p_const = ctx.enter_context(tc.tile_pool(name="const", bufs=1))
        p_kt = ctx.enter_context(tc.tile_pool(name="kt", bufs=NJB))
        p_qt = ctx.enter_context(tc.tile_pool(name="qt", bufs=NJB))
        p_v = ctx.enter_context(tc.tile_pool(name="v", bufs=NTK))
        p_xs = ctx.enter_context(tc.tile_pool(name="xs", bufs=8))
        p_ex = ctx.enter_context(tc.tile_pool(name="ex", bufs=6))
        p_ot = ctx.enter_context(tc.tile_pool(name="ot", bufs=8))
        p_rc = ctx.enter_context(tc.tile_pool(name="rc", bufs=6))
        p_st = ctx.enter_context(tc.tile_pool(name="st", bufs=2))
        # PSUM: scores 2x2 banks + av 2x1 + proj 2x1 = 8 banks
        p_sc = ctx.enter_context(tc.tile_pool(name="sc", bufs=2, space="PSUM"))
        p_av = ctx.enter_context(tc.tile_pool(name="av", bufs=2, space="PSUM"))
        p_pj = ctx.enter_context(tc.tile_pool(name="pj", bufs=1, space="PSUM"))
        p_bc = ctx.enter_context(tc.tile_pool(name="bc", bufs=1, space="PSUM"))

        # ---- constants ----
        w_q = p_const.tile([KB, NKB, HALF], BF16, tag="wq")
        nc.sync.dma_start(w_q[:], wqT.rearrange("(kb p) j -> p kb j", p=KB))
        w_k = p_const.tile([KB, NKB, HALF], BF16, tag="wk")
        nc.sync.dma_start(w_k[:], wkT.rearrange("(kb p) j -> p kb j", p=KB))
        w_v = p_const.tile([KB, NKB, HALF], BF16, tag="wv")
        nc.sync.dma_start(w_v[:], wvT.rearrange("(kb p) j -> p kb j", p=KB))
        w_o = p_const.tile([KB, NJB, D], BF16, tag="wo")
        nc.sync.dma_start(w_o[:], woT.rearrange("(jb p) n -> p jb n", p=KB))
        b_q = p_const.tile([KB, NJB], F32, tag="bq")
        nc.sync.dma_start(b_q[:], bq.rearrange("(jb p) one -> p (jb one)", p=KB))
        b_k = p_const.tile([KB, NJB], F32, tag="bk")
        nc.sync.dma_start(b_k[:], bk.rearrange("(jb p) one -> p (jb one)", p=KB))
        onesb = p_const.tile([DH + 1, DH + 1], BF16, tag="onesb")
        nc.vector.memset(onesb[:], 1.0)

        # ---- K^T / Q^T projections: {kt,qt}[jb] is [128 (j), T] bf16 ----
        kt_tiles = [p_kt.tile([KB, T], BF16, tag="kt", name=f"kt{j}") for j in range(NJB)]
        qt_tiles = [p_qt.tile([KB, T], BF16, tag="qt", name=f"qt{j}") for j in range(NJB)]
        for x_in, w_in, b_in, dst in (
            (xkT, w_k, b_k, kt_tiles),
            (xqT, w_q, b_q, qt_tiles),
        ):
            for tb in range(NTB):
                # 4 accumulators per tb: one 2-slot sc tile + two 1-bank pj
                # tiles; rings leave a full tb between reuse so the bias-add
                # drains overlap the next tb's matmuls.
                ps = p_sc.tile([KB, 2, TB], F32, tag="sc", name=f"psp{tb}")
                po_a = p_pj.tile([KB, TB], F32, tag="po", name=f"pop{tb}")
                po_b = p_bc.tile([KB, TB], F32, tag="bcr", name=f"pbp{tb}")
                targets = [ps[:, 0, :], ps[:, 1, :], po_a[:], po_b[:]]
                for kb in range(NKB):
                    xt = p_xs.tile([KB, TB], BF16, tag="xs")
                    nc.sync.dma_start(
                        xt[:],
                        x_in[kb * KB : (kb + 1) * KB, tb * TB : (tb + 1) * TB],
                    )
                    for jb in range(NJB):
                        nc.tensor.matmul(
                            targets[jb],
                            w_in[:, kb, jb * KB : (jb + 1) * KB],
                            xt[:],
                            start=(kb == 0),
                            stop=(kb == NKB - 1),
                        )
                for jb in range(NJB):
                    nc.vector.tensor_scalar_add(
                        dst[jb][:, tb * TB : (tb + 1) * TB],
                        targets[jb],
                        b_in[:, jb : jb + 1],
                    )

        # ---- V projection (natural layout): V[tk] is [128 (t), 8 (h), 65] ----
        # column 64 of each head is 1.0: the AV matmul then accumulates the
        # softmax denominator in psum row 64 for free.
        v_tiles = [
            p_v.tile([KB, H // 2, DH + 1], BF16, tag="v", name=f"v{j}")
            for j in range(NTK)
        ]
        for t in range(NTK):
            nc.vector.memset(v_tiles[t][:, :, DH : DH + 1], 1.0)
        for tb in range(NTB):
            ps = p_sc.tile([KB, 2, TB], F32, tag="sc", name=f"psv{tb}")
            po_a = p_pj.tile([KB, TB], F32, tag="po", name=f"pov{tb}")
            po_b = p_bc.tile([KB, TB], F32, tag="bcr", name=f"pbv{tb}")
            targets = [ps[:, 0, :], ps[:, 1, :], po_a[:], po_b[:]]
            for kb in range(NKB):
                xt = p_xs.tile([KB, TB], BF16, tag="xs")
                nc.sync.dma_start(
                    xt[:], xvT[kb * KB : (kb + 1) * KB, tb * TB : (tb + 1) * TB]
                )
                for ts in range(4):
                    nc.tensor.matmul(
                        targets[ts],
                        xt[:, ts * KB : (ts + 1) * KB],
                        w_v[:, kb, :],
                        start=(kb == 0),
                        stop=(kb == NKB - 1),
                    )
            for ts in range(4):
                nc.vector.tensor_copy(
                    v_tiles[tb * 4 + ts][:, :, 0:DH],
                    targets[ts].rearrange("p (h d) -> p h d", d=DH),
                )

        # ---- per t-block: attention + out-projection ----
        # Out-projection matmuls for t-block tq are emitted one per score
        # group during t-block tq+1 (fine-grained interleave, ~0.4us each).
        # Softmax normalization is split: AV-psum-freeing copies + denominator
        # reciprocals run at the head-pair boundary (DVE only, no DMAs); the
        # partition-crossing reciprocal broadcast (a K=1 bf16 matmul into a
        # dedicated 1-bank psum) and the scaling muls pop into the next
        # head-pair's early groups.
        def make_po_ops(ot_tiles, tq):
            ops = []
            for nb in range(2):
                for ts in range(4):
                    state = {}

                    def mk(jp, nb=nb, ts=ts, state=state):
                        def f():
                            if jp == 0:
                                state["po"] = p_pj.tile(
                                    [KB, TB], F32, tag="po", name=f"po{tq}_{nb}_{ts}"
                                )
                            nc.tensor.matmul(
                                state["po"][:],
                                ot_tiles[jp][:, ts * KB : (ts + 1) * KB],
                                w_o[:, jp, nb * TB : (nb + 1) * TB],
                                start=(jp == 0),
                                stop=(jp == NJB - 1),
                            )
                            if jp == NJB - 1:
                                st = p_st.tile(
                                    [KB, TB], F32, tag="st", name=f"st{tq}_{nb}_{ts}"
                                )
                                nc.vector.tensor_copy(st[:], state["po"][:])
                                nc.sync.dma_start(
                                    partial[
                                        tq * TB + ts * KB : tq * TB + (ts + 1) * KB,
                                        nb * TB : (nb + 1) * TB,
                                    ],
                                    st[:],
                                )

                        return f

                    for jp in range(NJB):
                        ops.append(mk(jp))
            return ops

        def mk_norm_ops(jp, asbs, rrbs, ot_tiles):
            state = {}

            def mk_bcast(i):
                def f():
                    # broadcast the raw denominator row across 64 partitions
                    t = p_bc.tile([KB, TB], F32, tag="bcr", name=f"bcr{i}")
                    state[i] = t[0:DH, :]
                    nc.tensor.matmul(
                        state[i],
                        onesb[DH : DH + 1, 0:DH],
                        rrbs[i][DH : DH + 1, :],
                        start=True,
                        stop=True,
                    )

                return f

            def mk_recip(i):
                def f():
                    # reciprocal at partition base 0 (custom-DVE op misbehaves
                    # on nonzero base); frees the bcr psum bank
                    rc2 = p_rc.tile([DH, TB], F32, tag="rc2", name=f"rc2{i}")
                    nc.vector.reciprocal_approx_fast(rc2[:], state[i])
                    state[2 + i] = rc2

                return f

            def mul0():
                nc.vector.tensor_mul(
                    ot_tiles[jp][0:DH, :], asbs[0][0:DH, :], state[2][:]
                )

            def mul1():
                # DVE can't shift partitions; stage then DMA into ot rows
                # 64:128 (gpsimd SWDGE ring, off the main DMA FIFO)
                stg = p_rc.tile([DH, TB], BF16, tag="stg")
                nc.vector.tensor_mul(stg[:], asbs[1][0:DH, :], state[3][:])
                nc.gpsimd.dma_start(ot_tiles[jp][DH : 2 * DH, :], stg[:])

            return [mk_bcast(0), mk_recip(0), mul0, mk_bcast(1), mk_recip(1), mul1]

        pending = []  # out-proj ops from the previous t-block
        norm_pending = []  # normalize tail ops from the previous head pair
        for tq in range(NTB):
            ot_tiles = [
                p_ot.tile([KB, TB], BF16, tag="ot", name=f"ot{tq}_{j}")
                for j in range(NJB)
            ]
            gctr = 0
            for jp in range(NJB):  # head pair (2*jp, 2*jp+1)
                avs = [
                    p_av.tile([DH + 1, TB], F32, tag="av", name=f"av{i}")
                    for i in range(2)
                ]
                for g in range(NTK):
                    sc = p_sc.tile([KB, 2, TB], F32, tag="sc")
                    for i in range(2):
                        nc.tensor.matmul(
                            sc[:, i, :],
                            kt_tiles[jp][i * DH : (i + 1) * DH, g * KB : (g + 1) * KB],
                            qt_tiles[jp][i * DH : (i + 1) * DH, tq * TB : (tq + 1) * TB],
                            start=True,
                            stop=True,
                        )
                    ex = p_ex.tile([KB, 2, TB], BF16, tag="ex")
                    nc.scalar.activation(
                        ex[:], sc[:], mybir.ActivationFunctionType.Exp, scale=0.125
                    )
                    for i in range(2):
                        nc.tensor.matmul(
                            avs[i][:],
                            v_tiles[g][:, 2 * jp + i, :],
                            ex[:, i, :],
                            start=(g == 0),
                            stop=(g == NTK - 1),
                        )
                    gctr += 1
                    if norm_pending and g >= 2:
                        norm_pending.pop(0)()
                    elif pending and gctr >= 10 and gctr % 2 == 0:
                        pending.pop(0)()
                # Boundary: free both AV psum banks with back-to-back copies,
                # then reciprocals of the denominator rows + bf16 casts for
                # the broadcast matmul. DVE-only; no DMAs anywhere upstream.
                asbs, rrbs = [], []
                for i in range(2):
                    asb = p_rc.tile([DH + 1, TB], F32, tag="asb", name=f"asb{i}")
                    nc.vector.tensor_copy(asb[:], avs[i][:])
                    asbs.append(asb)
                for i in range(2):
                    # bf16 cast of the raw denominator row (standard copy is
                    # fine at partition base 64; the custom reciprocal is NOT,
                    # so the reciprocal runs after the broadcast at base 0)
                    rrb = p_rc.tile([DH + 1, TB], BF16, tag="rrb", name=f"rrb{i}")
                    nc.vector.tensor_copy(rrb[DH : DH + 1, :], asbs[i][DH : DH + 1, :])
                    rrbs.append(rrb)
                norm_pending.extend(mk_norm_ops(jp, asbs, rrbs, ot_tiles))

            for op in pending:  # leftovers (pop schedule covers most)
                op()
            pending = make_po_ops(ot_tiles, tq)

        for op in norm_pending:
            op()
        for op in pending:
            op()

    nc.compile()
    return nc


def kernel(**inputs: np.ndarray) -> np.ndarray:
    query = np.asarray(inputs["query"], dtype=np.float32)
    key = np.asarray(inputs["key"], dtype=np.float32)
    value = np.asarray(inputs["value"], dtype=np.float32)
    w_q = np.asarray(inputs["w_q"], dtype=np.float32)
    b_q = np.asarray(inputs["b_q"], dtype=np.float32)
    w_k = np.asarray(inputs["w_k"], dtype=np.float32)
    b_k = np.asarray(inputs["b_k"], dtype=np.float32)
    w_v = np.asarray(inputs["w_v"], dtype=np.float32)
    b_v = np.asarray(inputs["b_v"], dtype=np.float32)
    w_o = np.asarray(inputs["w_o"], dtype=np.float32)
    b_o = np.asarray(inputs["b_o"], dtype=np.float32)

    nc = build_kernel()

    in_maps = []
    for c in range(N_CORES):
        b = c // 2
        hh = c % 2
        sl = slice(hh * HALF, (hh + 1) * HALF)
        in_maps.append(
            {
                "xqT": np.ascontiguousarray(query[b].T).astype(NPBF16),
                "xkT": np.ascontiguousarray(key[b].T).astype(NPBF16),
                "xvT": np.ascontiguousarray(value[b].T).astype(NPBF16),
                "wqT": np.ascontiguousarray(w_q[sl, :].T).astype(NPBF16),
                "wkT": np.ascontiguousarray(w_k[sl, :].T).astype(NPBF16),
                "wvT": np.ascontiguousarray(w_v[sl, :].T).astype(NPBF16),
                "woT": np.ascontiguousarray(w_o[:, sl].T).astype(NPBF16),
                "bq": np.ascontiguousarray(b_q[sl].reshape(HALF, 1)),
                "bk": np.ascontiguousarray(b_k[sl].reshape(HALF, 1)),
            }
        )

    res = run_bass_kernel_spmd(nc, in_maps, core_ids=list(range(N_CORES)))

    const_row = (b_v[None, :] @ w_o.T + b_o[None, :]).astype(np.float32)
    out = np.empty((B, T, D), dtype=np.float32)
    for b in range(B):
        out[b] = res.results[2 * b]["partial"] + res.results[2 * b + 1]["partial"]
        out[b] += const_row
    return out


# revision 22
# speedup vs baseline: 1.3474x; 1.0288x over previous
"""Multi-head attention (B=4, T=2048, D=1024, H=16) on 8 TRN2 NeuronCores.

Sharding: core c handles batch b = c//2 and head-half hh = c%2 (8 heads,
512 of the 1024 channel dims). Each core computes its half of the head
outputs and a row-sharded output projection, producing a partial
[T, D] output. Host unshard: out[b] = partial[2b] + partial[2b+1]
+ b_o + b_v @ w_o.T (the value-bias contribution commutes through
attention because softmax rows sum to 1).

All matmul operands are bf16 (hosts converts inputs); PSUM accumulates
fp32. Score PSUM is double-buffered ([128, 2x512] tiles) so the PE
computes the next group's scores while ACT runs exp on the previous
one, keeping the PE dense enough to hold its max p-state.
"""

from contextlib import ExitStack

import ml_dtypes
import numpy as np

import concourse.bass as bass
import concourse.mybir as mybir
import concourse.tile as tile
from concourse import bacc
from concourse.bass_utils import run_bass_kernel_spmd

B, T, D = 4, 2048, 1024
H = 16
DH = 64  # head dim
HALF = 512  # channels per core (8 heads)
N_CORES = 8

F32 = mybir.dt.float32
F32R = mybir.dt.float32r
BF16 = mybir.dt.bfloat16
NPBF16 = ml_dtypes.bfloat16

TB = 512  # t-block for moving operands
NTB = T // TB  # 4
KB = 128  # contraction block
NKB = D // KB  # 8
NJB = HALF // KB  # 4 j-blocks of the half
NTK = T // KB  # 16 tk blocks


def build_kernel():
    nc = bacc.Bacc(
        "TRN2", target_bir_lowering=False, debug=False, num_devices=N_CORES
    )
    xqT = nc.dram_tensor("xqT", [D, T], BF16, kind="ExternalInput").ap()
    xkT = nc.dram_tensor("xkT", [D, T], BF16, kind="ExternalInput").ap()
    xvT = nc.dram_tensor("xvT", [D, T], BF16, kind="ExternalInput").ap()
    wqT = nc.dram_tensor("wqT", [D, HALF], BF16, kind="ExternalInput").ap()
    wkT = nc.dram_tensor("wkT", [D, HALF], BF16, kind="ExternalInput").ap()
    wvT = nc.dram_tensor("wvT", [D, HALF], BF16, kind="ExternalInput").ap()
    woT = nc.dram_tensor("woT", [HALF, D], BF16, kind="ExternalInput").ap()
    bq = nc.dram_tensor("bq", [HALF, 1], F32, kind="ExternalInput").ap()
    bk = nc.dram_tensor("bk", [HALF, 1], F32, kind="ExternalInput").ap()
    partial = nc.dram_tensor("partial", [T, D], F32, kind="ExternalOutput").ap()

    with tile.TileContext(nc) as tc, ExitStack() as ctx:
        p_const = ctx.enter_context(tc.tile_pool(name="const", bufs=1))
        p_kt = ctx.enter_context(tc.tile_pool(name="kt", bufs=NJB))
        p_qt = ctx.enter_context(tc.tile_pool(name="qt", bufs=NJB))
        p_v = ctx.enter_context(tc.tile_pool(name="v", bufs=NTK))
        p_xs = ctx.enter_context(tc.tile_pool(name="xs", bufs=8))
        p_ex = ctx.enter_context(tc.tile_pool(name="ex", bufs=6))
        p_ot = ctx.enter_context(tc.tile_pool(name="ot", bufs=8))
        p_rc = ctx.enter_context(tc.tile_pool(name="rc", bufs=6))
        p_st = ctx.enter_context(tc.tile_pool(name="st", bufs=2))
        # PSUM: scores 2x2 banks + av 2x1 + proj 2x1 = 8 banks
        p_sc = ctx.enter_context(tc.tile_pool(name="sc", bufs=2, space="PSUM"))
        p_av = ctx.enter_context(tc.tile_pool(name="av", bufs=2, space="PSUM"))
        p_pj = ctx.enter_context(tc.tile_pool(name="pj", bufs=1, space="PSUM"))
        p_bc = ctx.enter_context(tc.tile_pool(name="bc", bufs=1, space="PSUM"))

        # ---- constants ----
        # w_k loads per-kb chunk so the first K-proj matmul starts after
        # ~1/8 of the weight traffic; the other weights follow behind.
        w_k = p_const.tile([KB, NKB, HALF], BF16, tag="wk")
        wkr = wkT.rearrange("(kb p) j -> p kb j", p=KB)
        for kb in range(NKB):
            nc.sync.dma_start(w_k[:, kb, :], wkr[:, kb, :])
        w_v = p_const.tile([KB, NKB, HALF], BF16, tag="wv")
        nc.sync.dma_start(w_v[:], wvT.rearrange("(kb p) j -> p kb j", p=KB))
        w_q = p_const.tile([KB, NKB, HALF], BF16, tag="wq")
        nc.sync.dma_start(w_q[:], wqT.rearrange("(kb p) j -> p kb j", p=KB))
        w_o = p_const.tile([KB, NJB, D], BF16, tag="wo")
        nc.sync.dma_start(w_o[:], woT.rearrange("(jb p) n -> p jb n", p=KB))
        b_q = p_const.tile([KB, NJB], F32, tag="bq")
        nc.sync.dma_start(b_q[:], bq.rearrange("(jb p) one -> p (jb one)", p=KB))
        b_k = p_const.tile([KB, NJB], F32, tag="bk")
        nc.sync.dma_start(b_k[:], bk.rearrange("(jb p) one -> p (jb one)", p=KB))
        onesb = p_const.tile([DH + 1, DH + 1], BF16, tag="onesb")
        nc.vector.memset(onesb[:], 1.0)

        # ---- K^T / Q^T projections: {kt,qt}[jb] is [128 (j), T] bf16 ----
        kt_tiles = [p_kt.tile([KB, T], BF16, tag="kt", name=f"kt{j}") for j in range(NJB)]
        qt_tiles = [p_qt.tile([KB, T], BF16, tag="qt", name=f"qt{j}") for j in range(NJB)]
        for x_in, w_in, b_in, dst in (
            (xkT, w_k, b_k, kt_tiles),
            (xqT, w_q, b_q, qt_tiles),
        ):
            for tb in range(NTB):
                # 4 accumulators per tb: one 2-slot sc tile + two 1-bank pj
                # tiles; rings leave a full tb between reuse so the bias-add
                # drains overlap the next tb's matmuls.
                ps = p_sc.tile([KB, 2, TB], F32, tag="sc", name=f"psp{tb}")
                po_a = p_pj.tile([KB, TB], F32, tag="po", name=f"pop{tb}")
                po_b = p_bc.tile([KB, TB], F32, tag="bcr", name=f"pbp{tb}")
                targets = [ps[:, 0, :], ps[:, 1, :], po_a[:], po_b[:]]
                for kb in range(NKB):
                    xt = p_xs.tile([KB, TB], BF16, tag="xs")
                    nc.sync.dma_start(
                        xt[:],
                        x_in[kb * KB : (kb + 1) * KB, tb * TB : (tb + 1) * TB],
                    )
                    for jb in range(NJB):
                        nc.tensor.matmul(
                            targets[jb],
                            w_in[:, kb, jb * KB : (jb + 1) * KB],
                            xt[:],
                            start=(kb == 0),
                            stop=(kb == NKB - 1),
                        )
                for jb in range(NJB):
                    nc.vector.tensor_scalar_add(
                        dst[jb][:, tb * TB : (tb + 1) * TB],
                        targets[jb],
                        b_in[:, jb : jb + 1],
                    )

        # ---- V projection (natural layout): V[tk] is [128 (t), 8 (h), 65] ----
        # column 64 of each head is 1.0: the AV matmul then accumulates the
        # softmax denominator in psum row 64 for free.
        v_tiles = [
            p_v.tile([KB, H // 2, DH + 1], BF16, tag="v", name=f"v{j}")
            for j in range(NTK)
        ]
        for t in range(NTK):
            nc.vector.memset(v_tiles[t][:, :, DH : DH + 1], 1.0)
        for tb in range(NTB):
            ps = p_sc.tile([KB, 2, TB], F32, tag="sc", name=f"psv{tb}")
            po_a = p_pj.tile([KB, TB], F32, tag="po", name=f"pov{tb}")
            po_b = p_bc.tile([KB, TB], F32, tag="bcr", name=f"pbv{tb}")
            targets = [ps[:, 0, :], ps[:, 1, :], po_a[:], po_b[:]]
            for kb in range(NKB):
                xt = p_xs.tile([KB, TB], BF16, tag="xs")
                nc.sync.dma_start(
                    xt[:], xvT[kb * KB : (kb + 1) * KB, tb * TB : (tb + 1) * TB]
                )
                for ts in range(4):
                    nc.tensor.matmul(
                        targets[ts],
                        xt[:, ts * KB : (ts + 1) * KB],
                        w_v[:, kb, :],
                        start=(kb == 0),
                        stop=(kb == NKB - 1),
                    )
            for ts in range(4):
                nc.vector.tensor_copy(
                    v_tiles[tb * 4 + ts][:, :, 0:DH],
                    targets[ts].rearrange("p (h d) -> p h d", d=DH),
                )

        # ---- per t-block: attention + out-projection ----
        # Out-projection matmuls for t-block tq are emitted one per score
        # group during t-block tq+1 (fine-grained interleave, ~0.4us each).
        # Softmax normalization is split: AV-psum-freeing copies + denominator
        # reciprocals run at the head-pair boundary (DVE only, no DMAs); the
        # partition-crossing reciprocal broadcast (a K=1 bf16 matmul into a
        # dedicated 1-bank psum) and the scaling muls pop into the next
        # head-pair's early groups.
        def make_po_ops(ot_tiles, tq, alt_banks=False):
            ops = []
            for ci, (nb, ts) in enumerate(
                (nb, ts) for nb in range(2) for ts in range(4)
            ):
                    state = {}
                    pool = p_bc if (alt_banks and ci % 2) else p_pj
                    ptag = "bcr" if (alt_banks and ci % 2) else "po"

                    def mk(jp, nb=nb, ts=ts, state=state, pool=pool, ptag=ptag):
                        def f():
                            if jp == 0:
                                state["po"] = pool.tile(
                                    [KB, TB], F32, tag=ptag, name=f"po{tq}_{nb}_{ts}"
                                )
                            nc.tensor.matmul(
                                state["po"][:],
                                ot_tiles[jp][:, ts * KB : (ts + 1) * KB],
                                w_o[:, jp, nb * TB : (nb + 1) * TB],
                                start=(jp == 0),
                                stop=(jp == NJB - 1),
                            )
                            if jp == NJB - 1:
                                st = p_st.tile(
                                    [KB, TB], F32, tag="st", name=f"st{tq}_{nb}_{ts}"
                                )
                                nc.vector.tensor_copy(st[:], state["po"][:])
                                nc.sync.dma_start(
                                    partial[
                                        tq * TB + ts * KB : tq * TB + (ts + 1) * KB,
                                        nb * TB : (nb + 1) * TB,
                                    ],
                                    st[:],
                                )

                        return f

                    for jp in range(NJB):
                        ops.append(mk(jp))
            return ops

        def mk_norm_ops(jp, asbs, rrbs, ot_tiles):
            state = {}

            def mk_bcast(i):
                def f():
                    # broadcast the raw denominator row across 64 partitions
                    t = p_bc.tile([KB, TB], F32, tag="bcr", name=f"bcr{i}")
                    state[i] = t[0:DH, :]
                    nc.tensor.matmul(
                        state[i],
                        onesb[DH : DH + 1, 0:DH],
                        rrbs[i][DH : DH + 1, :],
                        start=True,
                        stop=True,
                    )

                return f

            def mk_recip(i):
                def f():
                    # reciprocal at partition base 0 (custom-DVE op misbehaves
                    # on nonzero base); frees the bcr psum bank
                    rc2 = p_rc.tile([DH, TB], F32, tag="rc2", name=f"rc2{i}")
                    nc.vector.reciprocal_approx_fast(rc2[:], state[i])
                    state[2 + i] = rc2

                return f

            def mul0():
                nc.vector.tensor_mul(
                    ot_tiles[jp][0:DH, :], asbs[0][0:DH, :], state[2][:]
                )

            def mul1():
                # DVE can't shift partitions; stage then DMA into ot rows
                # 64:128 (gpsimd SWDGE ring, off the main DMA FIFO)
                stg = p_rc.tile([DH, TB], BF16, tag="stg")
                nc.vector.tensor_mul(stg[:], asbs[1][0:DH, :], state[3][:])
                nc.gpsimd.dma_start(ot_tiles[jp][DH : 2 * DH, :], stg[:])

            return [mk_bcast(0), mk_recip(0), mul0, mk_bcast(1), mk_recip(1), mul1]

        pending = []  # out-proj ops from the previous t-block
        norm_pending = []  # normalize tail ops from the previous head pair
        for tq in range(NTB):
            ot_tiles = [
                p_ot.tile([KB, TB], BF16, tag="ot", name=f"ot{tq}_{j}")
                for j in range(NJB)
            ]
            gctr = 0
            for jp in range(NJB):  # head pair (2*jp, 2*jp+1)
                avs = [
                    p_av.tile([DH + 1, TB], F32, tag="av", name=f"av{i}")
                    for i in range(2)
                ]
                for g in range(NTK):
                    sc = p_sc.tile([KB, 2, TB], F32, tag="sc")
                    for i in range(2):
                        nc.tensor.matmul(
                            sc[:, i, :],
                            kt_tiles[jp][i * DH : (i + 1) * DH, g * KB : (g + 1) * KB],
                            qt_tiles[jp][i * DH : (i + 1) * DH, tq * TB : (tq + 1) * TB],
                            start=True,
                            stop=True,
                        )
                    ex = p_ex.tile([KB, 2, TB], BF16, tag="ex")
                    nc.scalar.activation(
                        ex[:], sc[:], mybir.ActivationFunctionType.Exp, scale=0.125
                    )
                    for i in range(2):
                        nc.tensor.matmul(
                            avs[i][:],
                            v_tiles[g][:, 2 * jp + i, :],
                            ex[:, i, :],
                            start=(g == 0),
                            stop=(g == NTK - 1),
                        )
                    gctr += 1
                    if norm_pending and g >= 2:
                        norm_pending.pop(0)()
                    elif pending and gctr >= 10 and gctr % 2 == 0:
                        pending.pop(0)()
                # Boundary: free both AV psum banks with back-to-back copies,
                # then reciprocals of the denominator rows + bf16 casts for
                # the broadcast matmul. DVE-only; no DMAs anywhere upstream.
                asbs, rrbs = [], []
                for i in range(2):
                    asb = p_rc.tile([DH + 1, TB], F32, tag="asb", name=f"asb{i}")
                    nc.vector.tensor_copy(asb[:], avs[i][:])
                    asbs.append(asb)
                for i in range(2):
                    # bf16 cast of the raw denominator row (standard copy is
                    # fine at partition base 64; the custom reciprocal is NOT,
                    # so the reciprocal runs after the broadcast at base 0)
                    rrb = p_rc.tile([DH + 1, TB], BF16, tag="rrb", name=f"rrb{i}")
                    nc.vector.tensor_copy(rrb[DH : DH + 1, :], asbs[i][DH : DH + 1, :])
                    rrbs.append(rrb)
                norm_pending.extend(mk_norm_ops(jp, asbs, rrbs, ot_tiles))

            for op in pending:  # leftovers (pop schedule covers most)
                op()
            if tq < NTB - 1:
                pending = make_po_ops(ot_tiles, tq)
            else:
                pending = []
                final_ops = make_po_ops(ot_tiles, tq, alt_banks=True)

        for op in norm_pending:
            op()
        for op in pending:
            op()
        for op in final_ops:
            op()

    nc.compile()
    return nc


def kernel(**inputs: np.ndarray) -> np.ndarray:
    query = np.asarray(inputs["query"], dtype=np.float32)
    key = np.asarray(inputs["key"], dtype=np.float32)
    value = np.asarray(inputs["value"], dtype=np.float32)
    w_q = np.asarray(inputs["w_q"], dtype=np.float32)
    b_q = np.asarray(inputs["b_q"], dtype=np.float32)
    w_k = np.asarray(inputs["w_k"], dtype=np.float32)
    b_k = np.asarray(inputs["b_k"], dtype=np.float32)
    w_v = np.asarray(inputs["w_v"], dtype=np.float32)
    b_v = np.asarray(inputs["b_v"], dtype=np.float32)
    w_o = np.asarray(inputs["w_o"], dtype=np.float32)
    b_o = np.asarray(inputs["b_o"], dtype=np.float32)

    nc = build_kernel()

    in_maps = []
    for c in range(N_CORES):
        b = c // 2
        hh = c % 2
        sl = slice(hh * HALF, (hh + 1) * HALF)
        in_maps.append(
            {
                "xqT": np.ascontiguousarray(query[b].T).astype(NPBF16),
                "xkT": np.ascontiguousarray(key[b].T).astype(NPBF16),
                "xvT": np.ascontiguousarray(value[b].T).astype(NPBF16),
                "wqT": np.ascontiguousarray(w_q[sl, :].T).astype(NPBF16),
                "wkT": np.ascontiguousarray(w_k[sl, :].T).astype(NPBF16),
                "wvT": np.ascontiguousarray(w_v[sl, :].T).astype(NPBF16),
                "woT": np.ascontiguousarray(w_o[:, sl].T).astype(NPBF16),
                "bq": np.ascontiguousarray(b_q[sl].reshape(HALF, 1)),
                "bk": np.ascontiguousarray(b_k[sl].reshape(HALF, 1)),
            }
        )

    res = run_bass_kernel_spmd(nc, in_maps, core_ids=list(range(N_CORES)))

    const_row = (b_v[None, :] @ w_o.T + b_o[None, :]).astype(np.float32)
    out = np.empty((B, T, D), dtype=np.float32)
    for b in range(B):
        out[b] = res.results[2 * b]["partial"] + res.results[2 * b + 1]["partial"]
        out[b] += const_row
    return out


# revision 23
# speedup vs baseline: 1.3543x; 1.0051x over previous
"""Multi-head attention (B=4, T=2048, D=1024, H=16) on 8 TRN2 NeuronCores.

Sharding: core c handles batch b = c//2 and head-half hh = c%2 (8 heads,
512 of the 1024 channel dims). Each core computes its half of the head
outputs and a row-sharded output projection, producing a partial
[T, D] output. Host unshard: out[b] = partial[2b] + partial[2b+1]
+ b_o + b_v @ w_o.T (the value-bias contribution commutes through
attention because softmax rows sum to 1).

All matmul operands are bf16 (hosts converts inputs); PSUM accumulates
fp32. Score PSUM is double-buffered ([128, 2x512] tiles) so the PE
computes the next group's scores while ACT runs exp on the previous
one, keeping the PE dense enough to hold its max p-state.
"""

from contextlib import ExitStack

import ml_dtypes
import numpy as np

import concourse.bass as bass
import concourse.mybir as mybir
import concourse.tile as tile
from concourse import bacc
from concourse.bass_utils import run_bass_kernel_spmd

B, T, D = 4, 2048, 1024
H = 16
DH = 64  # head dim
HALF = 512  # channels per core (8 heads)
N_CORES = 8

F32 = mybir.dt.float32
F32R = mybir.dt.float32r
BF16 = mybir.dt.bfloat16
NPBF16 = ml_dtypes.bfloat16

TB = 512  # t-block for moving operands
NTB = T // TB  # 4
KB = 128  # contraction block
NKB = D // KB  # 8
NJB = HALF // KB  # 4 j-blocks of the half
NTK = T // KB  # 16 tk blocks


def build_kernel():
    nc = bacc.Bacc(
        "TRN2", target_bir_lowering=False, debug=False, num_devices=N_CORES
    )
    xqT = nc.dram_tensor("xqT", [D, T], BF16, kind="ExternalInput").ap()
    xkT = nc.dram_tensor("xkT", [D, T], BF16, kind="ExternalInput").ap()
    xvT = nc.dram_tensor("xvT", [D, T], BF16, kind="ExternalInput").ap()
    wqT = nc.dram_tensor("wqT", [D, HALF], BF16, kind="ExternalInput").ap()
    wkT = nc.dram_tensor("wkT", [D, HALF], BF16, kind="ExternalInput").ap()
    wvT = nc.dram_tensor("wvT", [D, HALF], BF16, kind="ExternalInput").ap()
    woT = nc.dram_tensor("woT", [HALF, D], BF16, kind="ExternalInput").ap()
    bq = nc.dram_tensor("bq", [HALF, 1], F32, kind="ExternalInput").ap()
    bk = nc.dram_tensor("bk", [HALF, 1], F32, kind="ExternalInput").ap()
    partial = nc.dram_tensor("partial", [T, D], F32, kind="ExternalOutput").ap()

    with tile.TileContext(nc) as tc, ExitStack() as ctx:
        p_const = ctx.enter_context(tc.tile_pool(name="const", bufs=1))
        p_kt = ctx.enter_context(tc.tile_pool(name="kt", bufs=NJB))
        p_qt = ctx.enter_context(tc.tile_pool(name="qt", bufs=NJB))
        p_v = ctx.enter_context(tc.tile_pool(name="v", bufs=NTK))
        p_xs = ctx.enter_context(tc.tile_pool(name="xs", bufs=8))
        p_ex = ctx.enter_context(tc.tile_pool(name="ex", bufs=6))
        p_ot = ctx.enter_context(tc.tile_pool(name="ot", bufs=8))
        p_rc = ctx.enter_context(tc.tile_pool(name="rc", bufs=6))
        p_st = ctx.enter_context(tc.tile_pool(name="st", bufs=2))
        # PSUM: scores 2x2 banks + av 2x1 + proj 2x1 = 8 banks
        p_sc = ctx.enter_context(tc.tile_pool(name="sc", bufs=2, space="PSUM"))
        p_av = ctx.enter_context(tc.tile_pool(name="av", bufs=2, space="PSUM"))
        p_pj = ctx.enter_context(tc.tile_pool(name="pj", bufs=1, space="PSUM"))
        p_bc = ctx.enter_context(tc.tile_pool(name="bc", bufs=1, space="PSUM"))

        # ---- constants ----
        # w_k loads per-kb chunk so the first K-proj matmul starts after
        # ~1/8 of the weight traffic; the other weights follow behind.
        w_k = p_const.tile([KB, NKB, HALF], BF16, tag="wk")
        wkr = wkT.rearrange("(kb p) j -> p kb j", p=KB)
        for kb in range(NKB):
            nc.sync.dma_start(w_k[:, kb, :], wkr[:, kb, :])
        w_v = p_const.tile([KB, NKB, HALF], BF16, tag="wv")
        w_q = p_const.tile([KB, NKB, HALF], BF16, tag="wq")
        w_o = p_const.tile([KB, NJB, D], BF16, tag="wo")
        b_q = p_const.tile([KB, NJB], F32, tag="bq")
        nc.sync.dma_start(b_q[:], bq.rearrange("(jb p) one -> p (jb one)", p=KB))
        b_k = p_const.tile([KB, NJB], F32, tag="bk")
        nc.sync.dma_start(b_k[:], bk.rearrange("(jb p) one -> p (jb one)", p=KB))
        onesb = p_const.tile([DH + 1, DH + 1], BF16, tag="onesb")
        nc.vector.memset(onesb[:], 1.0)

        # ---- K^T / Q^T projections: {kt,qt}[jb] is [128 (j), T] bf16 ----
        kt_tiles = [p_kt.tile([KB, T], BF16, tag="kt", name=f"kt{j}") for j in range(NJB)]
        qt_tiles = [p_qt.tile([KB, T], BF16, tag="qt", name=f"qt{j}") for j in range(NJB)]
        for pi, (x_in, w_in, b_in, dst) in enumerate((
            (xkT, w_k, b_k, kt_tiles),
            (xqT, w_q, b_q, qt_tiles),
        )):
            if pi == 1:
                # K-proj matmuls are queued; stream in the remaining weights
                # behind its x-tile loads.
                nc.sync.dma_start(w_q[:], wqT.rearrange("(kb p) j -> p kb j", p=KB))
                nc.sync.dma_start(w_v[:], wvT.rearrange("(kb p) j -> p kb j", p=KB))
                nc.sync.dma_start(w_o[:], woT.rearrange("(jb p) n -> p jb n", p=KB))
            for tb in range(NTB):
                # 4 accumulators per tb: one 2-slot sc tile + two 1-bank pj
                # tiles; rings leave a full tb between reuse so the bias-add
                # drains overlap the next tb's matmuls.
                ps = p_sc.tile([KB, 2, TB], F32, tag="sc", name=f"psp{tb}")
                po_a = p_pj.tile([KB, TB], F32, tag="po", name=f"pop{tb}")
                po_b = p_bc.tile([KB, TB], F32, tag="bcr", name=f"pbp{tb}")
                targets = [ps[:, 0, :], ps[:, 1, :], po_a[:], po_b[:]]
                for kb in range(NKB):
                    xt = p_xs.tile([KB, TB], BF16, tag="xs")
                    nc.sync.dma_start(
                        xt[:],
                        x_in[kb * KB : (kb + 1) * KB, tb * TB : (tb + 1) * TB],
                    )
                    for jb in range(NJB):
                        nc.tensor.matmul(
                            targets[jb],
                            w_in[:, kb, jb * KB : (jb + 1) * KB],
                            xt[:],
                            start=(kb == 0),
                            stop=(kb == NKB - 1),
                        )
                for jb in range(NJB):
                    nc.vector.tensor_scalar_add(
                        dst[jb][:, tb * TB : (tb + 1) * TB],
                        targets[jb],
                        b_in[:, jb : jb + 1],
                    )

        # ---- V projection (natural layout): V[tk] is [128 (t), 8 (h), 65] ----
        # column 64 of each head is 1.0: the AV matmul then accumulates the
        # softmax denominator in psum row 64 for free.
        v_tiles = [
            p_v.tile([KB, H // 2, DH + 1], BF16, tag="v", name=f"v{j}")
            for j in range(NTK)
        ]
        for t in range(NTK):
            nc.vector.memset(v_tiles[t][:, :, DH : DH + 1], 1.0)
        for tb in range(NTB):
            ps = p_sc.tile([KB, 2, TB], F32, tag="sc", name=f"psv{tb}")
            po_a = p_pj.tile([KB, TB], F32, tag="po", name=f"pov{tb}")
            po_b = p_bc.tile([KB, TB], F32, tag="bcr", name=f"pbv{tb}")
            targets = [ps[:, 0, :], ps[:, 1, :], po_a[:], po_b[:]]
            for kb in range(NKB):
                xt = p_xs.tile([KB, TB], BF16, tag="xs")
                nc.sync.dma_start(
                    xt[:], xvT[kb * KB : (kb + 1) * KB, tb * TB : (tb + 1) * TB]
                )
                for ts in range(4):
                    nc.tensor.matmul(
                        targets[ts],
                        xt[:, ts * KB : (ts + 1) * KB],
                        w_v[:, kb, :],
                        start=(kb == 0),
                        stop=(kb == NKB - 1),
                    )
            for ts in range(4):
                nc.vector.tensor_copy(
                    v_tiles[tb * 4 + ts][:, :, 0:DH],
                    targets[ts].rearrange("p (h d) -> p h d", d=DH),
                )

        # ---- per t-block: attention + out-projection ----
        # Out-projection matmuls for t-block tq are emitted one per score
        # group during t-block tq+1 (fine-grained interleave, ~0.4us each).
        # Softmax normalization is split: AV-psum-freeing copies + denominator
        # reciprocals run at the head-pair boundary (DVE only, no DMAs); the
        # partition-crossing reciprocal broadcast (a K=1 bf16 matmul into a
        # dedicated 1-bank psum) and the scaling muls pop into the next
        # head-pair's early groups.
        def make_po_ops(ot_tiles, tq, alt_banks=False):
            ops = []
            for ci, (nb, ts) in enumerate(
                (nb, ts) for nb in range(2) for ts in range(4)
            ):
                    state = {}
                    pool = p_bc if (alt_banks and ci % 2) else p_pj
                    ptag = "bcr" if (alt_banks and ci % 2) else "po"

                    def mk(jp, nb=nb, ts=ts, state=state, pool=pool, ptag=ptag):
                        def f():
                            if jp == 0:
                                state["po"] = pool.tile(
                                    [KB, TB], F32, tag=ptag, name=f"po{tq}_{nb}_{ts}"
                                )
                            nc.tensor.matmul(
                                state["po"][:],
                                ot_tiles[jp][:, ts * KB : (ts + 1) * KB],
                                w_o[:, jp, nb * TB : (nb + 1) * TB],
                                start=(jp == 0),
                                stop=(jp == NJB - 1),
                            )
                            if jp == NJB - 1:
                                st = p_st.tile(
                                    [KB, TB], F32, tag="st", name=f"st{tq}_{nb}_{ts}"
                                )
                                nc.vector.tensor_copy(st[:], state["po"][:])
                                nc.sync.dma_start(
                                    partial[
                                        tq * TB + ts * KB : tq * TB + (ts + 1) * KB,
                                        nb * TB : (nb + 1) * TB,
                                    ],
                                    st[:],
                                )

                        return f

                    for jp in range(NJB):
                        ops.append(mk(jp))
            return ops

        def mk_norm_ops(jp, asbs, rrbs, ot_tiles):
            state = {}

            def mk_bcast(i):
                def f():
                    # broadcast the raw denominator row across 64 partitions
                    t = p_bc.tile([KB, TB], F32, tag="bcr", name=f"bcr{i}")
                    state[i] = t[0:DH, :]
                    nc.tensor.matmul(
                        state[i],
                        onesb[DH : DH + 1, 0:DH],
                        rrbs[i][DH : DH + 1, :],
                        start=True,
                        stop=True,
                    )

                return f

            def mk_recip(i):
                def f():
                    # reciprocal at partition base 0 (custom-DVE op misbehaves
                    # on nonzero base); frees the bcr psum bank
                    rc2 = p_rc.tile([DH, TB], F32, tag="rc2", name=f"rc2{i}")
                    nc.vector.reciprocal_approx_fast(rc2[:], state[i])
                    state[2 + i] = rc2

                return f

            def mul0():
                nc.vector.tensor_mul(
                    ot_tiles[jp][0:DH, :], asbs[0][0:DH, :], state[2][:]
                )

            def mul1():
                # DVE can't shift partitions; stage then DMA into ot rows
                # 64:128 (gpsimd SWDGE ring, off the main DMA FIFO)
                stg = p_rc.tile([DH, TB], BF16, tag="stg")
                nc.vector.tensor_mul(stg[:], asbs[1][0:DH, :], state[3][:])
                nc.gpsimd.dma_start(ot_tiles[jp][DH : 2 * DH, :], stg[:])

            return [mk_bcast(0), mk_recip(0), mul0, mk_bcast(1), mk_recip(1), mul1]

        pending = []  # out-proj ops from the previous t-block
        norm_pending = []  # normalize tail ops from the previous head pair
        for tq in range(NTB):
            ot_tiles = [
                p_ot.tile([KB, TB], BF16, tag="ot", name=f"ot{tq}_{j}")
                for j in range(NJB)
            ]
            gctr = 0
            for jp in range(NJB):  # head pair (2*jp, 2*jp+1)
                avs = [
                    p_av.tile([DH + 1, TB], F32, tag="av", name=f"av{i}")
                    for i in range(2)
                ]
                for g in range(NTK):
                    sc = p_sc.tile([KB, 2, TB], F32, tag="sc")
                    for i in range(2):
                        nc.tensor.matmul(
                            sc[:, i, :],
                            kt_tiles[jp][i * DH : (i + 1) * DH, g * KB : (g + 1) * KB],
                            qt_tiles[jp][i * DH : (i + 1) * DH, tq * TB : (tq + 1) * TB],
                            start=True,
                            stop=True,
                        )
                    ex = p_ex.tile([KB, 2, TB], BF16, tag="ex")
                    nc.scalar.activation(
                        ex[:], sc[:], mybir.ActivationFunctionType.Exp, scale=0.125
                    )
                    for i in range(2):
                        nc.tensor.matmul(
                            avs[i][:],
                            v_tiles[g][:, 2 * jp + i, :],
                            ex[:, i, :],
                            start=(g == 0),
                            stop=(g == NTK - 1),
                        )
                    gctr += 1
                    if norm_pending and g >= 2:
                        norm_pending.pop(0)()
                    elif pending and gctr >= 10 and gctr % 2 == 0:
                        pending.pop(0)()
                # Boundary: free both AV psum banks with back-to-back copies,
                # then reciprocals of the denominator rows + bf16 casts for
                # the broadcast matmul. DVE-only; no DMAs anywhere upstream.
                asbs, rrbs = [], []
                for i in range(2):
                    asb = p_rc.tile([DH + 1, TB], F32, tag="asb", name=f"asb{i}")
                    nc.vector.tensor_copy(asb[:], avs[i][:])
                    asbs.append(asb)
                for i in range(2):
                    # bf16 cast of the raw denominator row (standard copy is
                    # fine at partition base 64; the custom reciprocal is NOT,
                    # so the reciprocal runs after the broadcast at base 0)
                    rrb = p_rc.tile([DH + 1, TB], BF16, tag="rrb", name=f"rrb{i}")
                    nc.vector.tensor_copy(rrb[DH : DH + 1, :], asbs[i][DH : DH + 1, :])
                    rrbs.append(rrb)
                norm_pending.extend(mk_norm_ops(jp, asbs, rrbs, ot_tiles))

            for op in pending:  # leftovers (pop schedule covers most)
                op()
            if tq < NTB - 1:
                pending = make_po_ops(ot_tiles, tq)
            else:
                pending = []
                final_ops = make_po_ops(ot_tiles, tq, alt_banks=True)

        for op in norm_pending:
            op()
        for op in pending:
            op()
        for op in final_ops:
            op()

    nc.compile()
    return nc


def kernel(**inputs: np.ndarray) -> np.ndarray:
    query = np.asarray(inputs["query"], dtype=np.float32)
    key = np.asarray(inputs["key"], dtype=np.float32)
    value = np.asarray(inputs["value"], dtype=np.float32)
    w_q = np.asarray(inputs["w_q"], dtype=np.float32)
    b_q = np.asarray(inputs["b_q"], dtype=np.float32)
    w_k = np.asarray(inputs["w_k"], dtype=np.float32)
    b_k = np.asarray(inputs["b_k"], dtype=np.float32)
    w_v = np.asarray(inputs["w_v"], dtype=np.float32)
    b_v = np.asarray(inputs["b_v"], dtype=np.float32)
    w_o = np.asarray(inputs["w_o"], dtype=np.float32)
    b_o = np.asarray(inputs["b_o"], dtype=np.float32)

    nc = build_kernel()

    in_maps = []
    for c in range(N_CORES):
        b = c // 2
        hh = c % 2
        sl = slice(hh * HALF, (hh + 1) * HALF)
        in_maps.append(
            {
                "xqT": np.ascontiguousarray(query[b].T).astype(NPBF16),
                "xkT": np.ascontiguousarray(key[b].T).astype(NPBF16),
                "xvT": np.ascontiguousarray(value[b].T).astype(NPBF16),
                "wqT": np.ascontiguousarray(w_q[sl, :].T).astype(NPBF16),
                "wkT": np.ascontiguousarray(w_k[sl, :].T).astype(NPBF16),
                "wvT": np.ascontiguousarray(w_v[sl, :].T).astype(NPBF16),
                "woT": np.ascontiguousarray(w_o[:, sl].T).astype(NPBF16),
                "bq": np.ascontiguousarray(b_q[sl].reshape(HALF, 1)),
                "bk": np.ascontiguousarray(b_k[sl].reshape(HALF, 1)),
            }
        )

    res = run_bass_kernel_spmd(nc, in_maps, core_ids=list(range(N_CORES)))

    const_row = (b_v[None, :] @ w_o.T + b_o[None, :]).astype(np.float32)
    out = np.empty((B, T, D), dtype=np.float32)
    for b in range(B):
        out[b] = res.results[2 * b]["partial"] + res.results[2 * b + 1]["partial"]
        out[b] += const_row
    return out
